# revision 41
# baseline (speedup 1.0000x reference)
"""CoAttLayer Trainium2 kernel.

Data-parallel over batch: 64 batches -> 8 NeuronCores x 8 batches.
Per batch (T = N = 1024, d = 64, k = 128):
    L  = tanh(R @ Wl @ P^T)                      (T, N)
    Hp = tanh(Wp @ P^T + (Wr @ R^T) @ L)         (k, N)
    Hr = tanh(Wr @ R^T + (Wp @ P^T) @ L^T)       (k, T)
    Ap = softmax(whp @ Hp), Ar = softmax(whr @ Hr)
    out = [P^T @ Ap ; R^T @ Ar]                  (2d,)

Layout strategy:
  * d-contractions (transposed R/P/A and small weights) live on partitions
    0-63; L tiles come out of PSUM t-major, tanh'd by ScalarE straight into
    fp8e4 SBUF.
  * The Hp and Hr accumulations over t/n (the two big 128-contraction
    passes) run as fp8e4 DoubleRow matmuls: two 128-row k-tiles per
    instruction at 0.5 cycles per moving column (4x the fp16 rate).
  * L^T is produced by the DMA xbar transpose operating on the fp8 L tiles
    viewed as fp16 byte-pairs.  Partition c of the transposed tile then
    holds the interleaved pair (n=2m, n=2m+1), m = 128g + c, which is
    exactly a DoubleRow k-tile pair.  The matching weights (Gp^T) are
    computed directly in that parity-packed layout with stride-2 lhsT
    slices of P^T, so no shuffle is ever needed.  This takes the big
    transpose off both the PE and the DVE.
"""

import numpy as np
from contextlib import ExitStack

B, T, N, D, K = 64, 1024, 1024, 64, 128
NCORES = 8
BL = B // NCORES  # batches per core

_CACHE = {}


def _build():
    import concourse.tile as tile
    from concourse import bacc, mybir
    from concourse.masks import make_identity

    f32 = mybir.dt.float32
    f32r = mybir.dt.float32r
    f16 = mybir.dt.float16
    f8 = mybir.dt.float8e4
    DR = mybir.MatmulPerfMode.DoubleRow
    Tanh = mybir.ActivationFunctionType.Tanh
    Exp = mybir.ActivationFunctionType.Exp

    nc = bacc.Bacc(trn_type="TRN2")

    rv = nc.dram_tensor("review_seq", (BL, T, D), f32r, kind="ExternalInput")
    po = nc.dram_tensor("post_seq", (BL, N, D), f32r, kind="ExternalInput")
    wl = nc.dram_tensor("Wl", (D, D), f32r, kind="ExternalInput")
    wr = nc.dram_tensor("Wr", (K, D), f32r, kind="ExternalInput")
    wp = nc.dram_tensor("Wp", (K, D), f32r, kind="ExternalInput")
    whr = nc.dram_tensor("whr", (1, K), f32, kind="ExternalInput")
    whp = nc.dram_tensor("whp", (1, K), f32, kind="ExternalInput")
    out = nc.dram_tensor("out", (BL, 2 * D), f32, kind="ExternalOutput")

    NT = T // 128  # 8 t-tiles
    NN = N // 128  # 8 n-tiles
    NG = N // 256  # 4 DoubleRow pair-chunks

    with tile.TileContext(nc) as tc, ExitStack() as ctx:
        singles = ctx.enter_context(tc.tile_pool(name="singles", bufs=1))
        sb = ctx.enter_context(tc.tile_pool(name="sb", bufs=2))
        # pa: 4 x 1-bank buffers (all pa tiles are <=2KB/partition) so the
        # ps_l rotation is 4 deep and the PE never waits on ScalarE tanh.
        pa = ctx.enter_context(tc.tile_pool(name="pa", bufs=4, space="PSUM"))
        pb = ctx.enter_context(tc.tile_pool(name="pb", bufs=2, space="PSUM"))

        # ---- per-core constants -------------------------------------------
        ident32 = singles.tile([128, 128], f32)
        make_identity(nc, ident32)
        ident = singles.tile([128, 128], f32r)
        nc.vector.tensor_copy(ident, ident32)
        one11 = singles.tile([1, 1], f32)
        nc.vector.memset(one11, 1.0)
        ident16 = singles.tile([128, 128], f16)
        nc.vector.tensor_copy(ident16, ident32)

        wl_sb = singles.tile([64, 64], f32r)
        nc.sync.dma_start(out=wl_sb, in_=wl[:, :])
        # [Wl | Wl]: A^T lands on PSUM rows 64:128 too, so B1 can consume
        # it at the same partition base as P^T.
        wl16d = singles.tile([64, 128], f16)
        nc.vector.tensor_copy(wl16d[:, 0:64], wl_sb)
        nc.vector.tensor_copy(wl16d[:, 64:128], wl_sb)
        wr_sb = singles.tile([128, 64], f32r)
        nc.sync.dma_start(out=wr_sb, in_=wr[:, :])
        wp_sb = singles.tile([128, 64], f32r)
        nc.sync.dma_start(out=wp_sb, in_=wp[:, :])
        whp_sb = singles.tile([1, 128], f32)
        nc.sync.dma_start(out=whp_sb, in_=whp[:, :])
        whr_sb = singles.tile([1, 128], f32)
        nc.sync.dma_start(out=whr_sb, in_=whr[:, :])

        # Wr^T on partitions 0-63 (R^T side); Wp^T moved to partitions
        # 64-127 (P^T side of the combined transpose) by a setup DMA.
        ps_w = pb.tile([128, 1024], f32r, tag="pb")
        nc.tensor.transpose(ps_w[0:64, 0:128], wr_sb, ident)
        nc.tensor.transpose(ps_w[0:64, 128:256], wp_sb, ident)
        wrT = singles.tile([64, 128], f16)
        nc.vector.tensor_copy(wrT, ps_w[0:64, 0:128])
        wpT0 = singles.tile([64, 128], f16)
        nc.vector.tensor_copy(wpT0, ps_w[0:64, 128:256])
        wpT64 = singles.tile([128, 128], f16)
        nc.sync.dma_start(out=wpT64[64:128, :], in_=wpT0[:, :])
        ps_wh = pb.tile([128, 2], f32, tag="pb")
        nc.tensor.transpose(ps_wh[0:128, 0:1], whp_sb, one11)
        nc.tensor.transpose(ps_wh[0:128, 1:2], whr_sb, one11)
        whT = singles.tile([128, 2], f16)
        nc.vector.tensor_copy(whT, ps_wh)
        # all per-batch outputs, stored once at the end
        obAll = singles.tile([2, BL, 129], f32)

        # ---- per-batch pipeline, software-pipelined emission ---------------
        # Emission per iteration k: A1(k+2) B1(k+1) A2(k+2) B2(k+1) A3(k+2)
        # B3(k+1) C(k).  The L^T DMA transposes issued inside B1(k+1) are
        # covered by the A2/B2/A3 emissions before B3(k+1) consumes them.
        st = {}

        def phaseA1(b):
            s = st[b] = {}
            # combined [R | P] tile: one DMA xbar transpose yields R^T on
            # partitions 0-63 and P^T on partitions 64-127.
            s["RPf"] = RPf = sb.tile(name="rpf", shape=[128, NT, 128], dtype=f32r, tag="rpf", bufs=3)
            # input loads ride the Activation hwdge queue; sharing the SP
            # queue with the transposes corrupts the transpose completion
            # ordering (seen as co_r errors on early batches).
            nc.scalar.dma_start(out=RPf[:, :, 0:64],
                                in_=rv[b, :, :].rearrange("(i p) d -> p i d", p=128))
            nc.scalar.dma_start(out=RPf[:, :, 64:128],
                                in_=po[b, :, :].rearrange("(i p) d -> p i d", p=128))
            RP16 = sb.tile(name="rp16", shape=[128, NT, 128], dtype=f16, tag="rp16", bufs=3)
            nc.vector.tensor_copy(RP16, RPf)
            s["RtPt"] = RtPt = sb.tile(name="rtpt", shape=[128, NT, 128], dtype=f16, tag="rtpt", bufs=4)
            nc.sync.dma_start_transpose(out=RtPt, in_=RP16[:, :, :])

        def phaseA2(b):
            s = st[b]
            RtPt = s["RtPt"]
            # A^T via [Wl | Wl]: rows 64:128 of PSUM carry the copy at the
            # P^T partition base for B1.
            s["AT"] = AT = sb.tile(name="at", shape=[128, 1024], dtype=f16, tag="at", bufs=3)
            for h in range(2):
                ps_at = pa.tile([128, 512], f32, tag="pa")
                nc.tensor.matmul(ps_at, wl16d, RtPt[0:64, 4 * h:4 * (h + 1), :],
                                 start=True, stop=True)
                nc.vector.tensor_copy(AT[64:128, 512 * h:512 * (h + 1)], ps_at[64:128, :])

        def phaseA3(b):
            s = st[b]
            RPf, RtPt = s["RPf"], s["RtPt"]
            # Gr^T tiles (Hp DoubleRow weights): Gr^T = R @ Wr^T, t-major.
            s["GTr"] = GTr = sb.tile(name="gtr", shape=[128, NT, 128], dtype=f8, tag="gtr", bufs=3)
            ps_gtr = pb.tile([128, NT, 128], f32, tag="pb")
            for a in range(NT):
                nc.tensor.matmul(ps_gtr[:, a, :], RtPt[0:64, a, :], wrT,
                                 start=True, stop=True)
            nc.vector.tensor_copy(GTr, ps_gtr)

            # Gp^T in parity-packed layout (Hr DoubleRow weights):
            # GTp[c, g, i, k] = Gp^T[256 g + 2 c + i, k], via stride-2 slices
            # of P^T as the stationary operand.
            s["GTp"] = GTp = sb.tile(name="gtp", shape=[128, NG, 2, 128], dtype=f8, tag="gtp", bufs=3)
            ps_gtp = pb.tile([128, NG, 2, 128], f32, tag="pb")
            for g in range(NG):
                PtI = RtPt[64:128, 2 * g:2 * g + 2, :].rearrange("d g (m i) -> d i g m", i=2)
                for par in range(2):
                    nc.tensor.matmul(ps_gtp[:, g, par, :],
                                     PtI[:, par, :, :], wpT64[64:128, :],
                                     start=True, stop=True)
            nc.vector.tensor_copy(GTp, ps_gtp)

            # fused pooling rhs [P | 1 | R] (gpsimd: off the DVE); the single
            # ones column yields both softmax denominators (row 0 = sum of
            # ee_p, row 1 = sum of ee_r).
            s["PRe"] = PRe = sb.tile(name="pre", shape=[128, NN, 129], dtype=f16, tag="pre", bufs=3)
            nc.gpsimd.tensor_copy(out=PRe[:, :, 0:64], in_=RPf[:, :, 64:128])
            nc.gpsimd.memset(PRe[:, :, 64:65], 1.0)
            nc.gpsimd.tensor_copy(out=PRe[:, :, 65:129], in_=RPf[:, :, 0:64])

        def phaseB1(b):
            s = st[b]
            RtPt, AT = s["RtPt"], s["AT"]
            # L tiles: L_i = tanh(A_i @ P^T) -> fp8 straight from ScalarE.
            # L^T via DMA xbar transpose of the fp8 pairs viewed as fp16:
            # LT16[c, g, t] <-> fp8 pair (n = 2(128g+c), n+1) at column t.
            s["Lf"] = Lf = sb.tile(name="lf", shape=[128, NT, 1024], dtype=f8, tag="lf", bufs=3)
            s["LT16"] = LT16 = sb.tile(name="lt16", shape=[128, NG, 1024], dtype=f16, tag="lt16", bufs=3)

            for i in range(NT):
                lhs = AT[64:128, 128 * i:128 * (i + 1)]
                for h in range(2):
                    ps_l = pa.tile([128, 512], f32, tag="pa")
                    nc.tensor.matmul(ps_l, lhs, RtPt[64:128, 4 * h:4 * (h + 1), :],
                                     start=True, stop=True)
                    nc.scalar.activation(Lf[:, i, 512 * h:512 * (h + 1)], ps_l, Tanh)
                nc.sync.dma_start_transpose(
                    out=LT16[:, :, 128 * i:128 * (i + 1)],
                    in_=Lf[:, i, :].bitcast(f16))

        def phaseB2(b):
            s = st[b]
            RtPt, GTr, Lf = s["RtPt"], s["GTr"], s["Lf"]
            # Hp = tanh(Wp @ P^T + sum_t Gr^T.T @ L), DoubleRow over t-tile
            # pairs.
            ps_hp = pb.tile([128, 1024], f32, tag="pb")
            nc.tensor.matmul(ps_hp[:, 0:512], wpT64[64:128, :], RtPt[64:128, 0:4, :],
                             start=True, stop=False)
            nc.tensor.matmul(ps_hp[:, 512:1024], wpT64[64:128, :], RtPt[64:128, 4:8, :],
                             start=True, stop=False)
            for a in range(NT // 2):
                last = a == NT // 2 - 1
                nc.tensor.matmul(ps_hp[:, 0:512], GTr[:, 2 * a:2 * a + 2, :],
                                 Lf[:, 2 * a:2 * a + 2, 0:512],
                                 start=False, stop=last, perf_mode=DR)
                nc.tensor.matmul(ps_hp[:, 512:1024], GTr[:, 2 * a:2 * a + 2, :],
                                 Lf[:, 2 * a:2 * a + 2, 512:1024],
                                 start=False, stop=last, perf_mode=DR)
            s["Hp16"] = Hp16 = sb.tile(name="hp16", shape=[128, 1024], dtype=f16, tag="hp16")
            nc.scalar.activation(Hp16, ps_hp, Tanh)

        def phaseB3(b):
            s = st[b]
            RtPt, GTp, LT16 = s["RtPt"], s["GTp"], s["LT16"]
            # Hr = tanh(Wr @ R^T + sum_n Gp^T.T @ L^T), DoubleRow over the
            # parity-packed pairs produced by the DMA transposes.
            ps_hr = pb.tile([128, 1024], f32, tag="pb")
            nc.tensor.matmul(ps_hr[:, 0:512], wrT, RtPt[0:64, 0:4, :], start=True, stop=False)
            nc.tensor.matmul(ps_hr[:, 512:1024], wrT, RtPt[0:64, 4:8, :], start=True, stop=False)
            for g in range(NG):
                last = g == NG - 1
                lhs = GTp[:, g, :, :]
                for h in range(2):
                    rhs = LT16[:, g, 512 * h:512 * (h + 1)].bitcast(f8) \
                        .rearrange("c (t i) -> c i t", i=2)
                    nc.tensor.matmul(ps_hr[:, 512 * h:512 * (h + 1)],
                                     lhs, rhs,
                                     start=False, stop=last, perf_mode=DR)
            s["Hr16"] = Hr16 = sb.tile(name="hr16", shape=[128, 1024], dtype=f16, tag="hr16")
            nc.scalar.activation(Hr16, ps_hr, Tanh)

        def phaseC1(b):
            s = st[b]
            Hp16, Hr16 = s["Hp16"], s["Hr16"]
            # logits^T: (n,1) and (t,1) per 128-chunk, then exp (no max-sub:
            # |logit| <= ||wh||_1 ~ 5, exp stays in fp16 range)
            ps_lg = pb.tile([128, 16], f32, tag="pb")
            for i in range(NN):
                nc.tensor.matmul(ps_lg[:, i:i + 1], Hp16[:, 128 * i:128 * (i + 1)],
                                 whT[:, 0:1], start=True, stop=True)
            for i in range(NT):
                nc.tensor.matmul(ps_lg[:, 8 + i:9 + i], Hr16[:, 128 * i:128 * (i + 1)],
                                 whT[:, 1:2], start=True, stop=True)
            s["ee"] = ee = sb.tile(name="ee", shape=[128, 16], dtype=f16, tag="ee")
            nc.scalar.activation(ee, ps_lg, Exp)

        def phaseC2(b):
            s = st.pop(b)
            PRe, ee = s["PRe"], s["ee"]
            # fused pooling: lhsT = (ee_p_j, ee_r_j), rhs = [P|1|R] ->
            # row 0 carries the P-side sums, row 1 the R-side (off-diagonal
            # quadrants are unused).
            eeR = ee[:, :].rearrange("p (s j) -> p j s", s=2)
            ps_co = pb.tile([2, 129], f32, tag="pb")
            for j in range(NN):
                nc.tensor.matmul(ps_co, eeR[:, j, :], PRe[:, j, :],
                                 start=(j == 0), stop=(j == NN - 1))

            rinv = sb.tile([2, 1], f32, tag="rinv")
            nc.vector.reciprocal(rinv, ps_co[0:2, 64:65])
            nc.vector.tensor_scalar_mul(obAll[:, b, :], ps_co[0:2, :], rinv)

        # A1's combined transpose is issued a full iteration before A2
        # consumes it; B1 (and its L^T DMA transposes) runs a full
        # iteration ahead of B3.  C(k) is emitted before B2/B3(k+1) so the
        # pb PSUM rotation never waits on a just-issued tanh.
        phaseA1(0)
        if BL > 1:
            phaseA1(1)
        phaseA2(0); phaseB1(0); phaseA3(0)
        if BL > 1:
            if BL > 2:
                phaseA1(2)
            phaseA2(1); phaseB1(1); phaseA3(1)
            phaseB2(0)
            phaseB3(0)
        else:
            phaseB2(0); phaseB3(0)
        for k in range(BL):
            if k + 3 < BL:
                phaseA1(k + 3)
            if k + 2 < BL:
                phaseA2(k + 2)
                phaseB1(k + 2)
                phaseA3(k + 2)
            phaseC1(k)
            phaseC2(k)
            if k + 1 < BL:
                phaseB2(k + 1)
                phaseB3(k + 1)
        nc.sync.dma_start(out=out[:, 0:64], in_=obAll[0:1, :, 0:64])
        nc.sync.dma_start(out=out[:, 64:128], in_=obAll[1:2, :, 65:129])

    nc.compile()
    return nc


def get_nc():
    if "nc" not in _CACHE:
        _CACHE["nc"] = _build()
    return _CACHE["nc"]


def make_in_maps(inputs):
    R = np.ascontiguousarray(inputs["review_seq"], dtype=np.float32)
    P = np.ascontiguousarray(inputs["post_seq"], dtype=np.float32)
    w = {
        "Wl": np.ascontiguousarray(inputs["Wl"], dtype=np.float32),
        "Wr": np.ascontiguousarray(inputs["Wr"], dtype=np.float32),
        "Wp": np.ascontiguousarray(inputs["Wp"], dtype=np.float32),
        "whr": np.ascontiguousarray(inputs["whr"], dtype=np.float32),
        "whp": np.ascontiguousarray(inputs["whp"], dtype=np.float32),
    }
    in_maps = []
    for c in range(NCORES):
        m = {
            "review_seq": np.ascontiguousarray(R[c * BL:(c + 1) * BL]),
            "post_seq": np.ascontiguousarray(P[c * BL:(c + 1) * BL]),
        }
        m.update(w)
        in_maps.append(m)
    return in_maps


def run(inputs, trace=False):
    from concourse.bass_utils import run_bass_kernel_spmd

    nc = get_nc()
    res = run_bass_kernel_spmd(nc, make_in_maps(inputs),
                               core_ids=list(range(NCORES)), trace=trace)
    outp = np.concatenate([r["out"] for r in res.results], axis=0)
    return outp.astype(np.float32), res


def kernel(**inputs) -> np.ndarray:
    outp, _ = run(inputs, trace=False)
    return outp


# revision 42
# speedup vs baseline: 1.0627x; 1.0627x over previous
"""CoAttLayer Trainium2 kernel.

Data-parallel over batch: 64 batches -> 8 NeuronCores x 8 batches.
Per batch (T = N = 1024, d = 64, k = 128):
    L  = tanh(R @ Wl @ P^T)                      (T, N)
    Hp = tanh(Wp @ P^T + (Wr @ R^T) @ L)         (k, N)
    Hr = tanh(Wr @ R^T + (Wp @ P^T) @ L^T)       (k, T)
    Ap = softmax(whp @ Hp), Ar = softmax(whr @ Hr)
    out = [P^T @ Ap ; R^T @ Ar]                  (2d,)

Layout strategy:
  * d-contractions (transposed R/P/A and small weights) live on partitions
    0-63; L tiles come out of PSUM t-major, tanh'd by ScalarE straight into
    fp8e4 SBUF.
  * The Hp and Hr accumulations over t/n (the two big 128-contraction
    passes) run as fp8e4 DoubleRow matmuls: two 128-row k-tiles per
    instruction at 0.5 cycles per moving column (4x the fp16 rate).
  * L^T is produced by the DMA xbar transpose operating on the fp8 L tiles
    viewed as fp16 byte-pairs.  Partition c of the transposed tile then
    holds the interleaved pair (n=2m, n=2m+1), m = 128g + c, which is
    exactly a DoubleRow k-tile pair.  The matching weights (Gp^T) are
    computed directly in that parity-packed layout with stride-2 lhsT
    slices of P^T, so no shuffle is ever needed.  This takes the big
    transpose off both the PE and the DVE.
"""

import numpy as np
from contextlib import ExitStack

B, T, N, D, K = 64, 1024, 1024, 64, 128
NCORES = 8
BL = B // NCORES  # batches per core

_CACHE = {}


def _build():
    import concourse.tile as tile
    from concourse import bacc, mybir
    from concourse.masks import make_identity

    f32 = mybir.dt.float32
    f32r = mybir.dt.float32r
    f16 = mybir.dt.float16
    f8 = mybir.dt.float8e4
    DR = mybir.MatmulPerfMode.DoubleRow
    Tanh = mybir.ActivationFunctionType.Tanh
    Exp = mybir.ActivationFunctionType.Exp

    nc = bacc.Bacc(trn_type="TRN2")

    rv = nc.dram_tensor("review_seq", (BL, T, D), f32r, kind="ExternalInput")
    po = nc.dram_tensor("post_seq", (BL, N, D), f32r, kind="ExternalInput")
    wl = nc.dram_tensor("Wl", (D, D), f32r, kind="ExternalInput")
    wr = nc.dram_tensor("Wr", (K, D), f32r, kind="ExternalInput")
    wp = nc.dram_tensor("Wp", (K, D), f32r, kind="ExternalInput")
    whr = nc.dram_tensor("whr", (1, K), f32, kind="ExternalInput")
    whp = nc.dram_tensor("whp", (1, K), f32, kind="ExternalInput")
    out = nc.dram_tensor("out", (BL, 2 * D), f32, kind="ExternalOutput")

    NT = T // 128  # 8 t-tiles
    NN = N // 128  # 8 n-tiles
    NG = N // 256  # 4 DoubleRow pair-chunks

    with tile.TileContext(nc) as tc, ExitStack() as ctx:
        singles = ctx.enter_context(tc.tile_pool(name="singles", bufs=1))
        sb = ctx.enter_context(tc.tile_pool(name="sb", bufs=2))
        # pa: 4 x 1-bank buffers (all pa tiles are <=2KB/partition) so the
        # ps_l rotation is 4 deep and the PE never waits on ScalarE tanh.
        pa = ctx.enter_context(tc.tile_pool(name="pa", bufs=4, space="PSUM"))
        pb = ctx.enter_context(tc.tile_pool(name="pb", bufs=2, space="PSUM"))

        # ---- per-core constants -------------------------------------------
        ident32 = singles.tile([128, 128], f32)
        make_identity(nc, ident32)
        ident = singles.tile([128, 128], f32r)
        nc.vector.tensor_copy(ident, ident32)
        one11 = singles.tile([1, 1], f32)
        nc.vector.memset(one11, 1.0)
        ident16 = singles.tile([128, 128], f16)
        nc.vector.tensor_copy(ident16, ident32)

        wl_sb = singles.tile([64, 64], f32r)
        nc.sync.dma_start(out=wl_sb, in_=wl[:, :])
        wl16 = singles.tile([64, 64], f16)
        nc.vector.tensor_copy(wl16, wl_sb)
        wr_sb = singles.tile([128, 64], f32r)
        nc.sync.dma_start(out=wr_sb, in_=wr[:, :])
        wp_sb = singles.tile([128, 64], f32r)
        nc.sync.dma_start(out=wp_sb, in_=wp[:, :])
        whp_sb = singles.tile([1, 128], f32)
        nc.sync.dma_start(out=whp_sb, in_=whp[:, :])
        whr_sb = singles.tile([1, 128], f32)
        nc.sync.dma_start(out=whr_sb, in_=whr[:, :])

        # Wr^T, Wp^T on partitions 0-63; whp^T/whr^T as fp16 columns.
        ps_w = pb.tile([128, 1024], f32r, tag="pb")
        nc.tensor.transpose(ps_w[0:64, 0:128], wr_sb, ident)
        nc.tensor.transpose(ps_w[0:64, 128:256], wp_sb, ident)
        wrT = singles.tile([64, 128], f16)
        nc.vector.tensor_copy(wrT, ps_w[0:64, 0:128])
        wpT = singles.tile([64, 128], f16)
        nc.vector.tensor_copy(wpT, ps_w[0:64, 128:256])
        ps_wh = pb.tile([128, 2], f32, tag="pb")
        nc.tensor.transpose(ps_wh[0:128, 0:1], whp_sb, one11)
        nc.tensor.transpose(ps_wh[0:128, 1:2], whr_sb, one11)
        whT = singles.tile([128, 2], f16)
        nc.vector.tensor_copy(whT, ps_wh)
        # all per-batch outputs, stored once at the end
        obAll = singles.tile([2, BL, 129], f32)

        # ---- per-batch pipeline, software-pipelined emission ---------------
        # Emission per iteration k: A1(k+2) B1(k+1) A2(k+2) B2(k+1) A3(k+2)
        # B3(k+1) C(k).  The L^T DMA transposes issued inside B1(k+1) are
        # covered by the A2/B2/A3 emissions before B3(k+1) consumes them.
        st = {}

        def phaseA1(b):
            s = st[b] = {}
            s["RP"] = RP = sb.tile(name="rp", shape=[128, NT, 64], dtype=f32r, tag="rp", bufs=3)
            s["PP"] = PP = sb.tile(name="pp", shape=[128, NN, 64], dtype=f32r, tag="pp", bufs=3)
            # input loads ride the Activation hwdge queue; sharing the SP
            # queue with the L^T transposes corrupts the transpose
            # completion ordering (seen as co_r errors on early batches).
            nc.scalar.dma_start(out=RP, in_=rv[b, :, :].rearrange("(i p) d -> p i d", p=128))
            nc.scalar.dma_start(out=PP, in_=po[b, :, :].rearrange("(i p) d -> p i d", p=128))

            s["R16"] = R16 = sb.tile(name="r16", shape=[128, NT, 64], dtype=f16, tag="r16", bufs=3)
            nc.vector.tensor_copy(R16, RP)
            s["P16"] = P16 = sb.tile(name="p16", shape=[128, NN, 64], dtype=f16, tag="p16", bufs=3)
            nc.vector.tensor_copy(P16, PP)

            ps_rt = pa.tile([128, 1024], f16, tag="pa", name="ps_rt")
            for i in range(NT):
                nc.tensor.transpose(ps_rt[0:64, 128 * i:128 * (i + 1)], R16[:, i, :], ident16)
            s["Rt"] = Rt = sb.tile(name="rt", shape=[64, 1024], dtype=f16, tag="rt", bufs=3)
            nc.vector.tensor_copy(Rt, ps_rt[0:64, :])

            ps_pt = pa.tile([128, 1024], f16, tag="pa", name="ps_pt")
            for i in range(NN):
                nc.tensor.transpose(ps_pt[0:64, 128 * i:128 * (i + 1)], P16[:, i, :], ident16)
            s["Pt"] = Pt = sb.tile(name="pt", shape=[64, 1024], dtype=f16, tag="pt", bufs=3)
            nc.vector.tensor_copy(Pt, ps_pt[0:64, :])

        def phaseA2(b):
            s = st[b]
            Rt = s["Rt"]
            s["AT"] = AT = sb.tile(name="at", shape=[64, 1024], dtype=f16, tag="at", bufs=3)
            for h in range(2):
                ps_at = pa.tile([64, 512], f32, tag="pa")
                nc.tensor.matmul(ps_at, wl16, Rt[:, 512 * h:512 * (h + 1)],
                                 start=True, stop=True)
                nc.vector.tensor_copy(AT[:, 512 * h:512 * (h + 1)], ps_at)

        def phaseA3(b):
            s = st[b]
            RP, PP = s["RP"], s["PP"]
            Rt, Pt = s["Rt"], s["Pt"]
            # Gr^T tiles (Hp DoubleRow weights): Gr^T = R @ Wr^T, t-major.
            s["GTr"] = GTr = sb.tile(name="gtr", shape=[128, NT, 128], dtype=f8, tag="gtr", bufs=3)
            ps_gtr = pb.tile([128, NT, 128], f32, tag="pb")
            for a in range(NT):
                nc.tensor.matmul(ps_gtr[:, a, :], Rt[:, 128 * a:128 * (a + 1)], wrT,
                                 start=True, stop=True)
            nc.vector.tensor_copy(GTr, ps_gtr)

            # Gp^T in parity-packed layout (Hr DoubleRow weights):
            # GTp[c, g, i, k] = Gp^T[256 g + 2 c + i, k], via stride-2 slices
            # of P^T as the stationary operand.
            s["GTp"] = GTp = sb.tile(name="gtp", shape=[128, NG, 2, 128], dtype=f8, tag="gtp", bufs=3)
            PtI = Pt[:, :].rearrange("d (m i) -> d i m", i=2)
            ps_gtp = pb.tile([128, NG, 2, 128], f32, tag="pb")
            for g in range(NG):
                for par in range(2):
                    nc.tensor.matmul(ps_gtp[:, g, par, :],
                                     PtI[:, par, 128 * g:128 * (g + 1)], wpT,
                                     start=True, stop=True)
            nc.vector.tensor_copy(GTp, ps_gtp)

            # fused pooling rhs [P | 1 | R] (gpsimd: off the DVE); the single
            # ones column yields both softmax denominators (row 0 = sum of
            # ee_p, row 1 = sum of ee_r).
            s["PRe"] = PRe = sb.tile(name="pre", shape=[128, NN, 129], dtype=f16, tag="pre", bufs=3)
            nc.gpsimd.tensor_copy(out=PRe[:, :, 0:64], in_=PP)
            nc.gpsimd.memset(PRe[:, :, 64:65], 1.0)
            nc.gpsimd.tensor_copy(out=PRe[:, :, 65:129], in_=RP)

        def phaseB1(b):
            s = st[b]
            Pt, AT = s["Pt"], s["AT"]
            # L tiles: L_i = tanh(A_i @ P^T) -> fp8 straight from ScalarE.
            # L^T via DMA xbar transpose of the fp8 pairs viewed as fp16:
            # LT16[c, g, t] <-> fp8 pair (n = 2(128g+c), n+1) at column t.
            s["Lf"] = Lf = sb.tile(name="lf", shape=[128, NT, 1024], dtype=f8, tag="lf", bufs=3)
            s["LT16"] = LT16 = sb.tile(name="lt16", shape=[128, NG, 1024], dtype=f16, tag="lt16", bufs=3)

            for i in range(NT):
                lhs = AT[:, 128 * i:128 * (i + 1)]
                for h in range(2):
                    ps_l = pa.tile([128, 512], f32, tag="pa")
                    nc.tensor.matmul(ps_l, lhs, Pt[:, 512 * h:512 * (h + 1)],
                                     start=True, stop=True)
                    nc.scalar.activation(Lf[:, i, 512 * h:512 * (h + 1)], ps_l, Tanh)
                nc.sync.dma_start_transpose(
                    out=LT16[:, :, 128 * i:128 * (i + 1)],
                    in_=Lf[:, i, :].bitcast(f16))

        def phaseB2(b):
            s = st[b]
            Pt, GTr, Lf = s["Pt"], s["GTr"], s["Lf"]
            # Hp = tanh(Wp @ P^T + sum_t Gr^T.T @ L), DoubleRow over t-tile
            # pairs.
            ps_hp = pb.tile([128, 1024], f32, tag="pb")
            nc.tensor.matmul(ps_hp[:, 0:512], wpT, Pt[:, 0:512], start=True, stop=False)
            nc.tensor.matmul(ps_hp[:, 512:1024], wpT, Pt[:, 512:1024], start=True, stop=False)
            for a in range(NT // 2):
                last = a == NT // 2 - 1
                nc.tensor.matmul(ps_hp[:, 0:512], GTr[:, 2 * a:2 * a + 2, :],
                                 Lf[:, 2 * a:2 * a + 2, 0:512],
                                 start=False, stop=last, perf_mode=DR)
                nc.tensor.matmul(ps_hp[:, 512:1024], GTr[:, 2 * a:2 * a + 2, :],
                                 Lf[:, 2 * a:2 * a + 2, 512:1024],
                                 start=False, stop=last, perf_mode=DR)
            s["Hp16"] = Hp16 = sb.tile(name="hp16", shape=[128, 1024], dtype=f16, tag="hp16")
            nc.scalar.activation(Hp16, ps_hp, Tanh)

        def phaseB3(b):
            s = st[b]
            Rt, GTp, LT16 = s["Rt"], s["GTp"], s["LT16"]
            # Hr = tanh(Wr @ R^T + sum_n Gp^T.T @ L^T), DoubleRow over the
            # parity-packed pairs produced by the DMA transposes.
            ps_hr = pb.tile([128, 1024], f32, tag="pb")
            nc.tensor.matmul(ps_hr[:, 0:512], wrT, Rt[:, 0:512], start=True, stop=False)
            nc.tensor.matmul(ps_hr[:, 512:1024], wrT, Rt[:, 512:1024], start=True, stop=False)
            for g in range(NG):
                last = g == NG - 1
                lhs = GTp[:, g, :, :]
                for h in range(2):
                    rhs = LT16[:, g, 512 * h:512 * (h + 1)].bitcast(f8) \
                        .rearrange("c (t i) -> c i t", i=2)
                    nc.tensor.matmul(ps_hr[:, 512 * h:512 * (h + 1)],
                                     lhs, rhs,
                                     start=False, stop=last, perf_mode=DR)
            s["Hr16"] = Hr16 = sb.tile(name="hr16", shape=[128, 1024], dtype=f16, tag="hr16")
            nc.scalar.activation(Hr16, ps_hr, Tanh)

        def phaseC1(b):
            s = st[b]
            Hp16, Hr16 = s["Hp16"], s["Hr16"]
            # logits^T: (n,1) and (t,1) per 128-chunk, then exp (no max-sub:
            # |logit| <= ||wh||_1 ~ 5, exp stays in fp16 range)
            ps_lg = pb.tile([128, 16], f32, tag="pb")
            for i in range(NN):
                nc.tensor.matmul(ps_lg[:, i:i + 1], Hp16[:, 128 * i:128 * (i + 1)],
                                 whT[:, 0:1], start=True, stop=True)
            for i in range(NT):
                nc.tensor.matmul(ps_lg[:, 8 + i:9 + i], Hr16[:, 128 * i:128 * (i + 1)],
                                 whT[:, 1:2], start=True, stop=True)
            s["ee"] = ee = sb.tile(name="ee", shape=[128, 16], dtype=f16, tag="ee")
            nc.scalar.activation(ee, ps_lg, Exp)

        def phaseC2(b):
            s = st.pop(b)
            PRe, ee = s["PRe"], s["ee"]
            # fused pooling: lhsT = (ee_p_j, ee_r_j), rhs = [P|1|R] ->
            # row 0 carries the P-side sums, row 1 the R-side (off-diagonal
            # quadrants are unused).
            eeR = ee[:, :].rearrange("p (s j) -> p j s", s=2)
            ps_co = pb.tile([2, 129], f32, tag="pb")
            for j in range(NN):
                nc.tensor.matmul(ps_co, eeR[:, j, :], PRe[:, j, :],
                                 start=(j == 0), stop=(j == NN - 1))

            rinv = sb.tile([2, 1], f32, tag="rinv")
            nc.vector.reciprocal(rinv, ps_co[0:2, 64:65])
            nc.vector.tensor_scalar_mul(obAll[:, b, :], ps_co[0:2, :], rinv)

        # B1 (and its L^T DMA transposes) runs a full iteration ahead of
        # B3, so each transpose has ~1.5 iterations of cover before its
        # consumer.  C(k) is emitted before B2/B3(k+1) so the pb PSUM
        # rotation never waits on a just-issued tanh.
        phaseA1(0); phaseA2(0); phaseB1(0); phaseA3(0)
        if BL > 1:
            phaseA1(1); phaseA2(1); phaseB1(1); phaseA3(1)
            phaseB2(0)
            phaseB3(0)
        else:
            phaseB2(0); phaseB3(0)
        for k in range(BL):
            if k + 2 < BL:
                phaseA1(k + 2)
                phaseA2(k + 2)
                phaseB1(k + 2)
                phaseA3(k + 2)
            phaseC1(k)
            phaseC2(k)
            if k + 1 < BL:
                phaseB2(k + 1)
                phaseB3(k + 1)
        nc.sync.dma_start(out=out[:, 0:64], in_=obAll[0:1, :, 0:64])
        nc.sync.dma_start(out=out[:, 64:128], in_=obAll[1:2, :, 65:129])

    nc.compile()
    return nc


def get_nc():
    if "nc" not in _CACHE:
        _CACHE["nc"] = _build()
    return _CACHE["nc"]


def make_in_maps(inputs):
    R = np.ascontiguousarray(inputs["review_seq"], dtype=np.float32)
    P = np.ascontiguousarray(inputs["post_seq"], dtype=np.float32)
    w = {
        "Wl": np.ascontiguousarray(inputs["Wl"], dtype=np.float32),
        "Wr": np.ascontiguousarray(inputs["Wr"], dtype=np.float32),
        "Wp": np.ascontiguousarray(inputs["Wp"], dtype=np.float32),
        "whr": np.ascontiguousarray(inputs["whr"], dtype=np.float32),
        "whp": np.ascontiguousarray(inputs["whp"], dtype=np.float32),
    }
    in_maps = []
    for c in range(NCORES):
        m = {
            "review_seq": np.ascontiguousarray(R[c * BL:(c + 1) * BL]),
            "post_seq": np.ascontiguousarray(P[c * BL:(c + 1) * BL]),
        }
        m.update(w)
        in_maps.append(m)
    return in_maps


def run(inputs, trace=False):
    from concourse.bass_utils import run_bass_kernel_spmd

    nc = get_nc()
    res = run_bass_kernel_spmd(nc, make_in_maps(inputs),
                               core_ids=list(range(NCORES)), trace=trace)
    outp = np.concatenate([r["out"] for r in res.results], axis=0)
    return outp.astype(np.float32), res


def kernel(**inputs) -> np.ndarray:
    outp, _ = run(inputs, trace=False)
    return outp


# revision 43
# speedup vs baseline: 1.0747x; 1.0113x over previous
"""CoAttLayer Trainium2 kernel.

Data-parallel over batch: 64 batches -> 8 NeuronCores x 8 batches.
Per batch (T = N = 1024, d = 64, k = 128):
    L  = tanh(R @ Wl @ P^T)                      (T, N)
    Hp = tanh(Wp @ P^T + (Wr @ R^T) @ L)         (k, N)
    Hr = tanh(Wr @ R^T + (Wp @ P^T) @ L^T)       (k, T)
    Ap = softmax(whp @ Hp), Ar = softmax(whr @ Hr)
    out = [P^T @ Ap ; R^T @ Ar]                  (2d,)

Layout strategy:
  * d-contractions (transposed R/P/A and small weights) live on partitions
    0-63; L tiles come out of PSUM t-major, tanh'd by ScalarE straight into
    fp8e4 SBUF.
  * The Hp and Hr accumulations over t/n (the two big 128-contraction
    passes) run as fp8e4 DoubleRow matmuls: two 128-row k-tiles per
    instruction (2x the fp16 contraction rate; measured 1 cycle per
    moving column on TRN2).
  * L^T is produced by the DMA xbar transpose operating on the fp8 L tiles
    viewed as fp16 byte-pairs.  Partition c of the transposed tile then
    holds the interleaved pair (n=2m, n=2m+1), m = 128g + c, which is
    exactly a DoubleRow k-tile pair.  The matching weights (Gp^T) are
    computed directly in that parity-packed layout with stride-2 lhsT
    slices of P^T, so no shuffle is ever needed.  This takes the big
    transpose off both the PE and the DVE.
"""

import numpy as np
from contextlib import ExitStack

B, T, N, D, K = 64, 1024, 1024, 64, 128
NCORES = 8
BL = B // NCORES  # batches per core

_CACHE = {}


def _build():
    import concourse.tile as tile
    from concourse import bacc, mybir
    from concourse.masks import make_identity

    f32 = mybir.dt.float32
    f32r = mybir.dt.float32r
    f16 = mybir.dt.float16
    f8 = mybir.dt.float8e4
    DR = mybir.MatmulPerfMode.DoubleRow
    Tanh = mybir.ActivationFunctionType.Tanh
    Exp = mybir.ActivationFunctionType.Exp

    nc = bacc.Bacc(trn_type="TRN2")

    rv = nc.dram_tensor("review_seq", (BL, T, D), f32r, kind="ExternalInput")
    po = nc.dram_tensor("post_seq", (BL, N, D), f32r, kind="ExternalInput")
    wl = nc.dram_tensor("Wl", (D, D), f32r, kind="ExternalInput")
    wr = nc.dram_tensor("Wr", (K, D), f32r, kind="ExternalInput")
    wp = nc.dram_tensor("Wp", (K, D), f32r, kind="ExternalInput")
    whr = nc.dram_tensor("whr", (1, K), f32, kind="ExternalInput")
    whp = nc.dram_tensor("whp", (1, K), f32, kind="ExternalInput")
    out = nc.dram_tensor("out", (BL, 2 * D), f32, kind="ExternalOutput")

    NT = T // 128  # 8 t-tiles
    NN = N // 128  # 8 n-tiles
    NG = N // 256  # 4 DoubleRow pair-chunks

    with tile.TileContext(nc) as tc, ExitStack() as ctx:
        singles = ctx.enter_context(tc.tile_pool(name="singles", bufs=1))
        sb = ctx.enter_context(tc.tile_pool(name="sb", bufs=2))
        # pa: 4 x 1-bank buffers (all pa tiles are <=2KB/partition) so the
        # ps_l rotation is 4 deep and the PE never waits on ScalarE tanh.
        pa = ctx.enter_context(tc.tile_pool(name="pa", bufs=4, space="PSUM"))
        pb = ctx.enter_context(tc.tile_pool(name="pb", bufs=2, space="PSUM"))

        # ---- per-core constants -------------------------------------------
        ident32 = singles.tile([128, 128], f32)
        make_identity(nc, ident32)
        ident = singles.tile([128, 128], f32r)
        nc.vector.tensor_copy(ident, ident32)
        one11 = singles.tile([1, 1], f32)
        nc.vector.memset(one11, 1.0)
        ident16 = singles.tile([128, 128], f16)
        nc.vector.tensor_copy(ident16, ident32)

        wl_sb = singles.tile([64, 64], f32r)
        nc.sync.dma_start(out=wl_sb, in_=wl[:, :])
        wl16 = singles.tile([64, 64], f16)
        nc.vector.tensor_copy(wl16, wl_sb)
        wr_sb = singles.tile([128, 64], f32r)
        nc.sync.dma_start(out=wr_sb, in_=wr[:, :])
        wp_sb = singles.tile([128, 64], f32r)
        nc.sync.dma_start(out=wp_sb, in_=wp[:, :])
        whp_sb = singles.tile([1, 128], f32)
        nc.sync.dma_start(out=whp_sb, in_=whp[:, :])
        whr_sb = singles.tile([1, 128], f32)
        nc.sync.dma_start(out=whr_sb, in_=whr[:, :])

        # Wr^T, Wp^T on partitions 0-63; whp^T/whr^T as fp16 columns.
        ps_w = pb.tile([128, 1024], f32r, tag="pb")
        nc.tensor.transpose(ps_w[0:64, 0:128], wr_sb, ident)
        nc.tensor.transpose(ps_w[0:64, 128:256], wp_sb, ident)
        wrT = singles.tile([64, 128], f16)
        nc.vector.tensor_copy(wrT, ps_w[0:64, 0:128])
        wpT = singles.tile([64, 128], f16)
        nc.vector.tensor_copy(wpT, ps_w[0:64, 128:256])
        ps_wh = pb.tile([128, 2], f32, tag="pb")
        nc.tensor.transpose(ps_wh[0:128, 0:1], whp_sb, one11)
        nc.tensor.transpose(ps_wh[0:128, 1:2], whr_sb, one11)
        whT = singles.tile([128, 2], f16)
        nc.vector.tensor_copy(whT, ps_wh)
        # all per-batch outputs, stored once at the end
        obAll = singles.tile([2, BL, 129], f32)

        # ---- per-batch pipeline, software-pipelined emission ---------------
        # Emission per iteration k: A1(k+2) B1(k+1) A2(k+2) B2(k+1) A3(k+2)
        # B3(k+1) C(k).  The L^T DMA transposes issued inside B1(k+1) are
        # covered by the A2/B2/A3 emissions before B3(k+1) consumes them.
        st = {}

        def phaseA1(b):
            s = st[b] = {}
            s["RP"] = RP = sb.tile(name="rp", shape=[128, NT, 64], dtype=f32r, tag="rp", bufs=3)
            s["PP"] = PP = sb.tile(name="pp", shape=[128, NN, 64], dtype=f32r, tag="pp", bufs=3)
            # input loads ride the Activation hwdge queue; sharing the SP
            # queue with the L^T transposes corrupts the transpose
            # completion ordering (seen as co_r errors on early batches).
            nc.scalar.dma_start(out=RP, in_=rv[b, :, :].rearrange("(i p) d -> p i d", p=128))
            nc.scalar.dma_start(out=PP, in_=po[b, :, :].rearrange("(i p) d -> p i d", p=128))

            s["R16"] = R16 = sb.tile(name="r16", shape=[128, NT, 64], dtype=f16, tag="r16", bufs=3)
            nc.vector.tensor_copy(R16, RP)
            s["P16"] = P16 = sb.tile(name="p16", shape=[128, NN, 64], dtype=f16, tag="p16", bufs=3)
            nc.vector.tensor_copy(P16, PP)

            ps_rt = pa.tile([128, 1024], f16, tag="pa", name="ps_rt")
            for i in range(NT):
                nc.tensor.transpose(ps_rt[0:64, 128 * i:128 * (i + 1)], R16[:, i, :], ident16)
            s["Rt"] = Rt = sb.tile(name="rt", shape=[64, 1024], dtype=f16, tag="rt", bufs=3)
            nc.vector.tensor_copy(Rt, ps_rt[0:64, :])

            ps_pt = pa.tile([128, 1024], f16, tag="pa", name="ps_pt")
            for i in range(NN):
                nc.tensor.transpose(ps_pt[0:64, 128 * i:128 * (i + 1)], P16[:, i, :], ident16)
            s["Pt"] = Pt = sb.tile(name="pt", shape=[64, 1024], dtype=f16, tag="pt", bufs=3)
            nc.vector.tensor_copy(Pt, ps_pt[0:64, :])

        def phaseA2(b):
            s = st[b]
            Rt = s["Rt"]
            s["AT"] = AT = sb.tile(name="at", shape=[64, 1024], dtype=f16, tag="at", bufs=3)
            for h in range(2):
                ps_at = pa.tile([64, 512], f32, tag="pa")
                nc.tensor.matmul(ps_at, wl16, Rt[:, 512 * h:512 * (h + 1)],
                                 start=True, stop=True)
                nc.vector.tensor_copy(AT[:, 512 * h:512 * (h + 1)], ps_at)

        def phaseA3(b):
            s = st[b]
            RP, PP = s["RP"], s["PP"]
            Rt, Pt = s["Rt"], s["Pt"]
            # Gr^T tiles (Hp DoubleRow weights): Gr^T = R @ Wr^T, t-major.
            s["GTr"] = GTr = sb.tile(name="gtr", shape=[128, NT, 128], dtype=f8, tag="gtr", bufs=3)
            ps_gtr = pb.tile([128, NT, 128], f32, tag="pb")
            for a in range(NT):
                nc.tensor.matmul(ps_gtr[:, a, :], Rt[:, 128 * a:128 * (a + 1)], wrT,
                                 start=True, stop=True)
            nc.vector.tensor_copy(GTr, ps_gtr)

            # Gp^T in parity-packed layout (Hr DoubleRow weights):
            # GTp[c, g, i, k] = Gp^T[256 g + 2 c + i, k], via stride-2 slices
            # of P^T as the stationary operand.
            s["GTp"] = GTp = sb.tile(name="gtp", shape=[128, NG, 2, 128], dtype=f8, tag="gtp", bufs=3)
            PtI = Pt[:, :].rearrange("d (m i) -> d i m", i=2)
            ps_gtp = pb.tile([128, NG, 2, 128], f32, tag="pb")
            for g in range(NG):
                for par in range(2):
                    nc.tensor.matmul(ps_gtp[:, g, par, :],
                                     PtI[:, par, 128 * g:128 * (g + 1)], wpT,
                                     start=True, stop=True)
            nc.vector.tensor_copy(GTp, ps_gtp)

            # fused pooling rhs [P | 1 | R] (gpsimd: off the DVE); the single
            # ones column yields both softmax denominators (row 0 = sum of
            # ee_p, row 1 = sum of ee_r).
            s["PRe"] = PRe = sb.tile(name="pre", shape=[128, NN, 129], dtype=f16, tag="pre", bufs=3)
            nc.gpsimd.tensor_copy(out=PRe[:, :, 0:64], in_=PP)
            nc.gpsimd.memset(PRe[:, :, 64:65], 1.0)
            nc.gpsimd.tensor_copy(out=PRe[:, :, 65:129], in_=RP)

        def phaseB1(b):
            s = st[b]
            Pt, AT = s["Pt"], s["AT"]
            # L tiles: L_i = tanh(A_i @ P^T) -> fp8 straight from ScalarE.
            # L^T via DMA xbar transpose of the fp8 pairs viewed as fp16:
            # LT16[c, g, t] <-> fp8 pair (n = 2(128g+c), n+1) at column t.
            s["Lf"] = Lf = sb.tile(name="lf", shape=[128, NT, 1024], dtype=f8, tag="lf", bufs=3)
            s["LT16"] = LT16 = sb.tile(name="lt16", shape=[128, NG, 1024], dtype=f16, tag="lt16", bufs=3)

            for i in range(NT):
                lhs = AT[:, 128 * i:128 * (i + 1)]
                for h in range(2):
                    ps_l = pa.tile([128, 512], f32, tag="pa")
                    nc.tensor.matmul(ps_l, lhs, Pt[:, 512 * h:512 * (h + 1)],
                                     start=True, stop=True)
                    nc.scalar.activation(Lf[:, i, 512 * h:512 * (h + 1)], ps_l, Tanh)
                nc.sync.dma_start_transpose(
                    out=LT16[:, :, 128 * i:128 * (i + 1)],
                    in_=Lf[:, i, :].bitcast(f16))

        def phaseB2(b):
            s = st[b]
            Pt, GTr, Lf = s["Pt"], s["GTr"], s["Lf"]
            # Hp = tanh(Wp @ P^T + sum_t Gr^T.T @ L), DoubleRow over t-tile
            # pairs.
            ps_hp = pb.tile([128, 1024], f32, tag="pb")
            nc.tensor.matmul(ps_hp[:, 0:512], wpT, Pt[:, 0:512], start=True, stop=False)
            nc.tensor.matmul(ps_hp[:, 512:1024], wpT, Pt[:, 512:1024], start=True, stop=False)
            for a in range(NT // 2):
                last = a == NT // 2 - 1
                nc.tensor.matmul(ps_hp[:, 0:512], GTr[:, 2 * a:2 * a + 2, :],
                                 Lf[:, 2 * a:2 * a + 2, 0:512],
                                 start=False, stop=last, perf_mode=DR)
                nc.tensor.matmul(ps_hp[:, 512:1024], GTr[:, 2 * a:2 * a + 2, :],
                                 Lf[:, 2 * a:2 * a + 2, 512:1024],
                                 start=False, stop=last, perf_mode=DR)
            s["Hp16"] = Hp16 = sb.tile(name="hp16", shape=[128, 1024], dtype=f16, tag="hp16")
            nc.scalar.activation(Hp16, ps_hp, Tanh)

        def phaseB3(b):
            s = st[b]
            Rt, GTp, LT16 = s["Rt"], s["GTp"], s["LT16"]
            # Hr = tanh(Wr @ R^T + sum_n Gp^T.T @ L^T), DoubleRow over the
            # parity-packed pairs produced by the DMA transposes.
            ps_hr = pb.tile([128, 1024], f32, tag="pb")
            nc.tensor.matmul(ps_hr[:, 0:512], wrT, Rt[:, 0:512], start=True, stop=False)
            nc.tensor.matmul(ps_hr[:, 512:1024], wrT, Rt[:, 512:1024], start=True, stop=False)
            for g in range(NG):
                last = g == NG - 1
                lhs = GTp[:, g, :, :]
                for h in range(2):
                    rhs = LT16[:, g, 512 * h:512 * (h + 1)].bitcast(f8) \
                        .rearrange("c (t i) -> c i t", i=2)
                    nc.tensor.matmul(ps_hr[:, 512 * h:512 * (h + 1)],
                                     lhs, rhs,
                                     start=False, stop=last, perf_mode=DR)
            s["Hr16"] = Hr16 = sb.tile(name="hr16", shape=[128, 1024], dtype=f16, tag="hr16")
            nc.scalar.activation(Hr16, ps_hr, Tanh)

        def phaseC1(b):
            s = st[b]
            Hp16, Hr16 = s["Hp16"], s["Hr16"]
            # logits^T: (n,1) and (t,1) per 128-chunk, then exp (no max-sub:
            # |logit| <= ||wh||_1 ~ 5, exp stays in fp16 range)
            ps_lg = pb.tile([128, 16], f32, tag="pb")
            for i in range(NN):
                nc.tensor.matmul(ps_lg[:, i:i + 1], Hp16[:, 128 * i:128 * (i + 1)],
                                 whT[:, 0:1], start=True, stop=True)
            for i in range(NT):
                nc.tensor.matmul(ps_lg[:, 8 + i:9 + i], Hr16[:, 128 * i:128 * (i + 1)],
                                 whT[:, 1:2], start=True, stop=True)
            s["ee"] = ee = sb.tile(name="ee", shape=[128, 16], dtype=f16, tag="ee")
            nc.scalar.activation(ee, ps_lg, Exp)

        def phaseC2(b):
            s = st.pop(b)
            PRe, ee = s["PRe"], s["ee"]
            # fused pooling: lhsT = (ee_p_j, ee_r_j), rhs = [P|1|R] ->
            # row 0 carries the P-side sums, row 1 the R-side (off-diagonal
            # quadrants are unused).
            eeR = ee[:, :].rearrange("p (s j) -> p j s", s=2)
            ps_co = pb.tile([2, 129], f32, tag="pb")
            for j in range(NN):
                nc.tensor.matmul(ps_co, eeR[:, j, :], PRe[:, j, :],
                                 start=(j == 0), stop=(j == NN - 1))

            rinv = sb.tile([2, 1], f32, tag="rinv")
            nc.vector.reciprocal(rinv, ps_co[0:2, 64:65])
            nc.vector.tensor_scalar_mul(obAll[:, b, :], ps_co[0:2, :], rinv)

        # B1 (and its L^T DMA transposes) runs a full iteration ahead of
        # B3, so each transpose has ~1.5 iterations of cover before its
        # consumer.  C(k) is emitted before B2/B3(k+1) so the pb PSUM
        # rotation never waits on a just-issued tanh.
        phaseA1(0); phaseA2(0); phaseB1(0); phaseA3(0)
        if BL > 1:
            phaseA1(1); phaseA2(1); phaseB1(1); phaseA3(1)
            phaseB2(0)
            phaseB3(0)
        else:
            phaseB2(0); phaseB3(0)
        for k in range(BL):
            if k + 2 < BL:
                phaseA1(k + 2)
                phaseA2(k + 2)
                phaseB1(k + 2)
                phaseA3(k + 2)
            phaseC1(k)
            phaseC2(k)
            if k + 1 < BL:
                phaseB2(k + 1)
                phaseB3(k + 1)
        nc.sync.dma_start(out=out[:, 0:64], in_=obAll[0:1, :, 0:64])
        nc.sync.dma_start(out=out[:, 64:128], in_=obAll[1:2, :, 65:129])

    nc.compile()
    return nc


def get_nc():
    if "nc" not in _CACHE:
        _CACHE["nc"] = _build()
    return _CACHE["nc"]


def make_in_maps(inputs):
    R = np.ascontiguousarray(inputs["review_seq"], dtype=np.float32)
    P = np.ascontiguousarray(inputs["post_seq"], dtype=np.float32)
    w = {
        "Wl": np.ascontiguousarray(inputs["Wl"], dtype=np.float32),
        "Wr": np.ascontiguousarray(inputs["Wr"], dtype=np.float32),
        "Wp": np.ascontiguousarray(inputs["Wp"], dtype=np.float32),
        "whr": np.ascontiguousarray(inputs["whr"], dtype=np.float32),
        "whp": np.ascontiguousarray(inputs["whp"], dtype=np.float32),
    }
    in_maps = []
    for c in range(NCORES):
        m = {
            "review_seq": np.ascontiguousarray(R[c * BL:(c + 1) * BL]),
            "post_seq": np.ascontiguousarray(P[c * BL:(c + 1) * BL]),
        }
        m.update(w)
        in_maps.append(m)
    return in_maps


def run(inputs, trace=False):
    from concourse.bass_utils import run_bass_kernel_spmd

    nc = get_nc()
    res = run_bass_kernel_spmd(nc, make_in_maps(inputs),
                               core_ids=list(range(NCORES)), trace=trace)
    outp = np.concatenate([r["out"] for r in res.results], axis=0)
    return outp.astype(np.float32), res


def kernel(**inputs) -> np.ndarray:
    outp, _ = run(inputs, trace=False)
    return outp
